# revision 14
# baseline (speedup 1.0000x reference)
"""Single-head memory attention on Trainium2, batch-parallel across 8 NeuronCores.

Per core (one batch element):
    Q^T = Wq @ x^T + bq                  (MM1, bf16, fp32 accum)
    S^T = keys @ Q^T                     (MM2; k on partitions, q on free dim)
    E^T = exp(S^T/sqrt(d) + mask_k)      (one ScalarE activation: scale+bias+exp)
    sums= ones^T @ (DVE-accumulated E)   (denominator: 15 DVE adds + 1 matmul)
    O   = E^T.T @ V  * recip(sums)       (MM3 + per-partition normalize)

All transposes are plain matmuls against an identity/ones moving operand
(transpose-mode PE ops don't count as HAM activity and left the PE at
1.2 GHz through the staging phase). A burst of throwaway matmuls on a
scratch tile fills the DMA-bound opening so the HAM clock gate flips to
2.4 GHz ~15us earlier. Partition-scatter loads (bq, mask, per-chunk softmax
sums) are N=1 matmul-transposes instead of 4-byte-descriptor gather DMAs,
which monopolized the Sync engine's DMA-issue pipeline for ~5us each.

Values are DMA'd in dv-halves and MM3 runs dv-major, so the dv=1 half
converts behind the dv=0 chains; every engine-FIFO consumer is emitted
after its producer's queue position to avoid cross-engine priority
inversions (DVE is strict FIFO). Output stores are bf16.
"""

import numpy as np

import concourse.bacc as bacc
import concourse.mybir as mybir
from concourse.tile import TileContext
from concourse.masks import make_identity
from concourse.bass_utils import run_bass_kernel_spmd

B, LQ, LK, D = 8, 2048, 2048, 1024
P = 128
QCH = 512                 # queries processed per chunk
NQC = LQ // QCH           # 4 chunks
NDT = D // P              # 8 tiles along d (contraction of MM1)
NET = D // P              # 8 tiles along e (contraction of MM2)
NKT = LK // P             # 16 tiles along k (contraction of MM3)
NQS = QCH // P            # 4 query subtiles per chunk
SCALE = 1.0 / float(np.sqrt(D))

F32 = mybir.dt.float32
BF16 = mybir.dt.bfloat16
AFT = mybir.ActivationFunctionType

_CACHE = {}


def build_nc():
    nc = bacc.Bacc(None, target_bir_lowering=False)

    x_d = nc.dram_tensor("x", [LQ, D], F32, kind="ExternalInput")
    keys_d = nc.dram_tensor("keys", [LK, D], F32, kind="ExternalInput")
    values_d = nc.dram_tensor("values", [LK, D], F32, kind="ExternalInput")
    mask_d = nc.dram_tensor("mask", [LK, 1], F32, kind="ExternalInput")
    wq_d = nc.dram_tensor("Wq", [D, D], F32, kind="ExternalInput")
    bq_d = nc.dram_tensor("bq", [D], F32, kind="ExternalInput")
    out_d = nc.dram_tensor("out", [LQ, D], BF16, kind="ExternalOutput")

    with TileContext(nc) as tc:
        with (
            tc.tile_pool(name="persist", bufs=1) as persist,
            tc.tile_pool(name="stage", bufs=6) as stagep,
            tc.tile_pool(name="cvt", bufs=4) as cvtp,
            tc.tile_pool(name="vstage", bufs=3) as vstagep,
            tc.tile_pool(name="xTp", bufs=2) as xTp,
            tc.tile_pool(name="QTp", bufs=2) as QTp,
            tc.tile_pool(name="ETp", bufs=2) as ETp,
            tc.tile_pool(name="osb", bufs=3) as osbp,
            tc.tile_pool(name="sums", bufs=2) as sumsp,
            tc.tile_pool(name="accp", bufs=2) as accp,
            tc.tile_pool(name="psT", bufs=2, space="PSUM") as psTp,
            tc.tile_pool(name="psAcc", bufs=3, space="PSUM") as psAccp,
            tc.tile_pool(name="psS", bufs=1, space="PSUM") as psSp,
        ):
            # ---- constants ----
            ident = persist.tile([P, P], BF16)
            make_identity(nc, ident)
            ones_f32 = persist.tile([P, 1], F32)
            nc.any.memset(ones_f32, 1.0)
            garb = persist.tile([P, QCH], BF16)
            nc.any.memset(garb, 0.25)
            bq_sb = persist.tile([P, NDT], F32)
            mask_sb = persist.tile([P, NKT], F32)
            bq_row = persist.tile([1, D], F32)
            mask_row = persist.tile([1, LK], F32)

            # ---- persistent operands ----
            WqT = persist.tile([P, NDT, D], BF16)    # [d%P, d//P, e] = Wq[e, d]
            keysT = persist.tile([P, NET, LK], BF16)  # [e%P, e//P, k] = keys[k, e]
            Vsb = persist.tile([P, NKT, D], BF16)    # [k%P, k//P, dv] = values[k, dv]

            copy_eng = [
                lambda o, i: nc.vector.tensor_copy(o, i),
                lambda o, i: nc.scalar.copy(o, i),
            ]
            state = {"n": 0}

            pdum = psAccp.tile([P, QCH], F32, tag="acc", name="pdum")

            def dummy_burst(n):
                # throwaway matmuls: trip the HAM clock gate to 2.4 GHz
                # during the DMA-bound opening (PE would otherwise idle)
                for _ in range(n):
                    nc.tensor.matmul(pdum, garb[:, 0:P], garb,
                                     start=True, stop=True)

            def stage_rows(dram_rows, parity):
                st = stagep.tile([P, D], F32, tag="stage")
                nc.sync.dma_start(st, dram_rows)
                cv = cvtp.tile([P, D], BF16, tag="cvt")
                cvt = nc.vector.tensor_copy if parity % 2 == 0 else nc.scalar.copy
                cvt(cv, st)
                return cv

            def transpose_block(dst3, col0, cv):
                # dst3[:, ft, col0:col0+P] = cv[:, ft*P:(ft+1)*P].T for ft in
                # 0..7 as 8 plain matmuls (cv_block.T @ I); fp32 PSUM (2
                # banks), drained by one strided converting copy.
                pt = psTp.tile([P, NDT, P], F32, tag="pst")
                for ft in range(NDT):
                    nc.tensor.matmul(
                        pt[:, ft, :], cv[:, ft * P:(ft + 1) * P], ident,
                        start=True, stop=True,
                    )
                copy_eng[state["n"] % 2](dst3[:, :, col0:col0 + P], pt)
                state["n"] += 1

            def row_scatter(dst, row, nblk, name):
                # dst[p, t] = row[0, t*P + p] via nblk N=1 matmul-transposes
                ps = psSp.tile([P, nblk], F32, tag="pscat", name=name)
                for t in range(nblk):
                    nc.tensor.matmul(
                        ps[:, t:t + 1], row[0:1, t * P:(t + 1) * P],
                        ones_f32[0:1, 0:1], start=True, stop=True,
                    )
                nc.vector.tensor_copy(dst, ps)

            def mm1_chain(QT, xT, et):
                pq = psAccp.tile([P, QCH], F32, tag="acc")
                for dt in range(NDT):
                    nc.tensor.matmul(
                        pq,
                        WqT[:, dt, et * P:(et + 1) * P],
                        xT[:, dt, :],
                        start=(dt == 0),
                        stop=(dt == NDT - 1),
                    )
                nc.vector.tensor_scalar_add(QT[:, et, :], pq, bq_sb[:, et:et + 1])

            def mm2_chain(ET, QT, acc, kt):
                ps = psAccp.tile([P, QCH], F32, tag="acc")
                for et in range(NET):
                    nc.tensor.matmul(
                        ps,
                        keysT[:, et, kt * P:(kt + 1) * P],
                        QT[:, et, :],
                        start=(et == 0),
                        stop=(et == NET - 1),
                    )
                nc.scalar.activation(
                    ET[:, kt, :], ps, AFT.Exp,
                    bias=mask_sb[:, kt:kt + 1], scale=SCALE,
                )
                # denominator accumulation rides along on DVE
                if kt == 0:
                    nc.vector.tensor_copy(acc, ET[:, 0, :])
                else:
                    nc.vector.tensor_add(acc, acc, ET[:, kt, :])

            def sums_start(acc, c):
                # ones^T @ acc -> [1, QCH] in PSUM, copied to SBUF
                pd = psAccp.tile([1, QCH], F32, tag="acc", name=f"pd{c}")
                nc.tensor.matmul(pd, ones_f32, acc, start=True, stop=True)
                sums_sb = sumsp.tile([1, QCH], F32, tag="sums", name=f"ssb{c}")
                nc.vector.tensor_copy(sums_sb, pd)
                return sums_sb

            def sums_scatter(sums_sb, c):
                # [1, QCH] -> [P, NQS] via 4 N=1 matmul-transposes, then recip
                ps = psSp.tile([P, NQS], F32, tag="pscat", name=f"psc{c}")
                for t in range(NQS):
                    nc.tensor.matmul(
                        ps[:, t:t + 1], sums_sb[0:1, t * P:(t + 1) * P],
                        ones_f32[0:1, 0:1], start=True, stop=True,
                    )
                rc = sumsp.tile([P, NQS], F32, tag="rc", name=f"rc{c}")
                nc.vector.reciprocal(rc, ps)
                return rc

            def mm3_matmuls(ET, qs, dv):
                po = psAccp.tile([P, QCH], F32, tag="acc")
                for kt in range(NKT):
                    nc.tensor.matmul(
                        po,
                        ET[:, kt, qs * P:(qs + 1) * P],
                        Vsb[:, kt, dv * QCH:(dv + 1) * QCH],
                        start=(kt == 0),
                        stop=(kt == NKT - 1),
                    )
                return po

            def mm3_drain(qc, po, rc, qs, dv):
                osb = osbp.tile([P, QCH], BF16, tag="osb")
                if (dv * NQS + qs) % 2 == 0:
                    nc.vector.tensor_scalar_mul(osb, po, rc[:, qs:qs + 1])
                else:
                    nc.scalar.activation(
                        osb, po, AFT.Copy, bias=0.0, scale=rc[:, qs:qs + 1],
                    )
                nc.sync.dma_start(
                    out_d[qc * QCH + qs * P: qc * QCH + (qs + 1) * P,
                          dv * QCH:(dv + 1) * QCH],
                    osb,
                )

            def v_stage_half(kt, dv):
                st = vstagep.tile([P, QCH], F32, tag="vstage",
                                  name=f"vst{kt}_{dv}")
                nc.sync.dma_start(
                    st,
                    values_d[kt * P:(kt + 1) * P, dv * QCH:(dv + 1) * QCH],
                )
                return st

            def v_cvt_half(st, kt, dv):
                cvt = nc.vector.tensor_copy if (kt + dv) % 2 == 0 else nc.scalar.copy
                cvt(Vsb[:, kt, dv * QCH:(dv + 1) * QCH], st)

            # ---- emission ----
            # DMA order: x0, bq/mask rows (single-descriptor), Wq, x1, keys,
            # values dv0/dv1 halves, x2 (mid MM3(0)), x3 (mid MM3(1)).
            cv_x0 = [stage_rows(x_d[qs * P:(qs + 1) * P, :], qs)
                     for qs in range(NQS)]
            nc.sync.dma_start(bq_row, bq_d[:].rearrange("(o f) -> o f", o=1))
            nc.sync.dma_start(
                mask_row, mask_d[:].rearrange("(o f) z -> o (f z)", o=1)
            )
            cv_wq = [stage_rows(wq_d[et * P:(et + 1) * P, :], et)
                     for et in range(NDT)]
            cv_x1 = [stage_rows(x_d[QCH + qs * P:QCH + (qs + 1) * P, :], qs)
                     for qs in range(NQS)]
            cv_k = [stage_rows(keys_d[kt * P:(kt + 1) * P, :], kt)
                    for kt in range(NKT)]
            v_st = {}
            for kt in range(NKT):
                v_st[(kt, 0)] = v_stage_half(kt, 0)
            for kt in range(NKT):
                v_st[(kt, 1)] = v_stage_half(kt, 1)

            # PE: dummy warm-up bursts between the x0 transposes, bq/mask
            # scatters, then Wq transposes interleaved with MM1(0)
            xT0 = xTp.tile([P, NDT, QCH], BF16, tag="xT")
            dummy_burst(8)
            for qs in range(NQS):
                transpose_block(xT0, qs * P, cv_x0[qs])
                dummy_burst(5)
            row_scatter(bq_sb, bq_row, NDT, "bqscat")
            row_scatter(mask_sb, mask_row, NKT, "maskscat")
            dummy_burst(5)
            QT0 = QTp.tile([P, NET, QCH], BF16, tag="QT")
            for et in range(NDT):
                transpose_block(WqT, et * P, cv_wq[et])
                mm1_chain(QT0, xT0, et)

            # x1 transposes, then keys transposes interleaved with MM1(1)
            xT1 = xTp.tile([P, NDT, QCH], BF16, tag="xT")
            for qs in range(NQS):
                transpose_block(xT1, qs * P, cv_x1[qs])
            QT1 = QTp.tile([P, NET, QCH], BF16, tag="QT")
            for kt in range(NKT):
                transpose_block(keysT, kt * P, cv_k[kt])
                if kt % 2 == 1:
                    mm1_chain(QT1, xT1, kt // 2)

            QTs = [QT0, QT1, None, None]
            xTs = [xT0, xT1, None, None]

            # steady state per chunk c: MM2(c) with chunk-0 values dv0-half
            # cvts threaded into chains 8..15, then MM3(c) dv-major: the
            # denominator matmul follows the first chain, its scatter the
            # second; x(c+2) stages emit after the scatter and transpose on
            # the dv=1 chains; chunk-0 dv1-half cvts hide behind dv0 chains.
            for c in range(NQC):
                ET = ETp.tile([P, NKT, QCH], BF16, tag="ET", name=f"ET{c}")
                acc = accp.tile([P, QCH], F32, tag="accd", name=f"accd{c}")
                for kt in range(NKT):
                    mm2_chain(ET, QTs[c], acc, kt)
                    if c == 0 and kt >= 8:
                        v_cvt_half(v_st[(2 * (kt - 8), 0)], 2 * (kt - 8), 0)
                        v_cvt_half(v_st[(2 * (kt - 8) + 1, 0)],
                                   2 * (kt - 8) + 1, 0)
                rc = None
                cv_xn = None
                pend = []
                for dv in range(2):
                    for qs in range(NQS):
                        idx = dv * NQS + qs
                        if cv_xn is not None and dv == 1:
                            transpose_block(xTs[c + 2], qs * P, cv_xn[qs])
                        po = mm3_matmuls(ET, qs, dv)
                        if idx == 0:
                            sums_sb = sums_start(acc, c)
                            pend.append((po, qs, dv))
                            continue
                        if idx == 1:
                            rc = sums_scatter(sums_sb, c)
                            if c + 2 < NQC:
                                cv_xn = [stage_rows(
                                    x_d[(c + 2) * QCH + q2 * P:
                                        (c + 2) * QCH + (q2 + 1) * P, :], q2)
                                    for q2 in range(NQS)]
                                xTs[c + 2] = xTp.tile(
                                    [P, NDT, QCH], BF16, tag="xT",
                                    name=f"xTn{c}")
                            for ppo, pqs, pdv in pend:
                                mm3_drain(c, ppo, rc, pqs, pdv)
                            pend = []
                        if c == 0 and dv == 0 and idx >= 1:
                            for j in range(5 if idx < 3 else 6):
                                kt2 = (idx - 1) * 5 + j
                                if kt2 < NKT:
                                    v_cvt_half(v_st[(kt2, 1)], kt2, 1)
                        mm3_drain(c, po, rc, qs, dv)
                if c + 2 < NQC:
                    QTs[c + 2] = QTp.tile([P, NET, QCH], BF16, tag="QT",
                                          name=f"QTn{c}")
                    for et in range(NDT):
                        mm1_chain(QTs[c + 2], xTs[c + 2], et)

    nc.finalize()
    return nc


def _get_nc():
    if "nc" not in _CACHE:
        _CACHE["nc"] = build_nc()
    return _CACHE["nc"]


def kernel(x, mem_padding_mask, keys, values, Wq, bq):
    nc = _get_nc()
    Wq_c = np.ascontiguousarray(Wq, dtype=np.float32)
    bq_c = np.ascontiguousarray(bq, dtype=np.float32)
    in_maps = [
        {
            "x": np.ascontiguousarray(x[b], dtype=np.float32),
            "keys": np.ascontiguousarray(keys[b], dtype=np.float32),
            "values": np.ascontiguousarray(values[b], dtype=np.float32),
            "mask": np.ascontiguousarray(mem_padding_mask[b], dtype=np.float32),
            "Wq": Wq_c,
            "bq": bq_c,
        }
        for b in range(B)
    ]
    res = run_bass_kernel_spmd(nc, in_maps, core_ids=list(range(B)))
    return np.stack(
        [np.asarray(res.results[i]["out"]) for i in range(B)], axis=0
    ).astype(np.float32)


# revision 20
# speedup vs baseline: 1.0664x; 1.0664x over previous
"""Single-head memory attention on Trainium2, batch-parallel across 8 NeuronCores.

Per core (one batch element):
    Q^T = Wq @ x^T + bq                  (MM1, bf16, fp32 accum)
    S^T = keys @ Q^T                     (MM2; k on partitions, q on free dim)
    E^T = exp(S^T/sqrt(d) + mask_k)      (one ScalarE activation: scale+bias+exp)
    sums= ones^T @ (DVE-accumulated E)   (denominator: 15 DVE adds + 1 matmul)
    O   = E^T.T @ V  * recip(sums)       (MM3 + per-partition normalize)

Operand transposes (x^T, keys^T, Wq^T) are plain 128x128 matmuls against an
identity moving operand (NOT transpose-mode): transpose-mode PE ops don't
register as activity for the HAM clock gate, which kept the PE at 1.2 GHz
through the entire staging phase. Regular matmuls keep the array warm.

Emission interleaves the staging transposes and the first two chunks' MM1
chains with the Wq/keys DMA arrival so the PE never starves during the
28 MB input load, and MM1 runs two chunks ahead of MM2/MM3 thereafter.
"""

import numpy as np

import concourse.bacc as bacc
import concourse.mybir as mybir
from concourse.tile import TileContext
from concourse.masks import make_identity
from concourse.bass_utils import run_bass_kernel_spmd

B, LQ, LK, D = 8, 2048, 2048, 1024
P = 128
QCH = 512                 # queries processed per chunk
NQC = LQ // QCH           # 4 chunks
NDT = D // P              # 8 tiles along d (contraction of MM1)
NET = D // P              # 8 tiles along e (contraction of MM2)
NKT = LK // P             # 16 tiles along k (contraction of MM3)
NQS = QCH // P            # 4 query subtiles per chunk
SCALE = 1.0 / float(np.sqrt(D))

F32 = mybir.dt.float32
BF16 = mybir.dt.bfloat16
AFT = mybir.ActivationFunctionType

_CACHE = {}


def build_nc():
    nc = bacc.Bacc(None, target_bir_lowering=False)

    x_d = nc.dram_tensor("x", [LQ, D], F32, kind="ExternalInput")
    keys_d = nc.dram_tensor("keys", [LK, D], F32, kind="ExternalInput")
    values_d = nc.dram_tensor("values", [LK, D], F32, kind="ExternalInput")
    mask_d = nc.dram_tensor("mask", [LK, 1], F32, kind="ExternalInput")
    wq_d = nc.dram_tensor("Wq", [D, D], F32, kind="ExternalInput")
    bq_d = nc.dram_tensor("bq", [D], F32, kind="ExternalInput")
    out_d = nc.dram_tensor("out", [LQ, D], F32, kind="ExternalOutput")

    with TileContext(nc) as tc:
        with (
            tc.tile_pool(name="persist", bufs=1) as persist,
            tc.tile_pool(name="stage", bufs=6) as stagep,
            tc.tile_pool(name="cvt", bufs=4) as cvtp,
            tc.tile_pool(name="xTp", bufs=2) as xTp,
            tc.tile_pool(name="QTp", bufs=2) as QTp,
            tc.tile_pool(name="ETp", bufs=2) as ETp,
            tc.tile_pool(name="osb", bufs=3) as osbp,
            tc.tile_pool(name="sums", bufs=2) as sumsp,
            tc.tile_pool(name="accp", bufs=2) as accp,
            tc.tile_pool(name="psT", bufs=2, space="PSUM") as psTp,
            tc.tile_pool(name="psAcc", bufs=3, space="PSUM") as psAccp,
            tc.tile_pool(name="psD", bufs=1, space="PSUM") as psDp,
            tc.tile_pool(name="dram", bufs=2, space="DRAM") as dramp,
        ):
            # ---- constants ----
            ident = persist.tile([P, P], BF16)
            make_identity(nc, ident)
            ones_f32 = persist.tile([P, 1], F32)
            nc.any.memset(ones_f32, 1.0)
            bq_sb = persist.tile([P, NDT], F32)
            mask_sb = persist.tile([P, NKT], F32)

            # ---- persistent operands ----
            WqT = persist.tile([P, NDT, D], BF16)    # [d%P, d//P, e] = Wq[e, d]
            keysT = persist.tile([P, NET, LK], BF16)  # [e%P, e//P, k] = keys[k, e]
            Vsb = persist.tile([P, NKT, D], BF16)    # [k%P, k//P, dv] = values[k, dv]

            copy_eng = [
                lambda o, i: nc.vector.tensor_copy(o, i),
                lambda o, i: nc.scalar.copy(o, i),
            ]
            state = {"n": 0}

            def stage_rows(dram_rows, parity):
                st = stagep.tile([P, D], F32, tag="stage")
                nc.sync.dma_start(st, dram_rows)
                cv = cvtp.tile([P, D], BF16, tag="cvt")
                cvt = nc.vector.tensor_copy if parity % 2 == 0 else nc.scalar.copy
                cvt(cv, st)
                return cv

            def transpose_block(dst3, col0, cv):
                # dst3[:, ft, col0:col0+P] = cv[:, ft*P:(ft+1)*P].T for ft in
                # 0..7 as 8 plain matmuls (cv_block.T @ I); fp32 PSUM (2
                # banks), drained by one strided converting copy.
                pt = psTp.tile([P, NDT, P], F32, tag="pst")
                for ft in range(NDT):
                    nc.tensor.matmul(
                        pt[:, ft, :], cv[:, ft * P:(ft + 1) * P], ident,
                        start=True, stop=True,
                    )
                copy_eng[state["n"] % 2](dst3[:, :, col0:col0 + P], pt)
                state["n"] += 1

            def mm1_chain(QT, xT, et):
                pq = psAccp.tile([P, QCH], F32, tag="acc")
                for dt in range(NDT):
                    nc.tensor.matmul(
                        pq,
                        WqT[:, dt, et * P:(et + 1) * P],
                        xT[:, dt, :],
                        start=(dt == 0),
                        stop=(dt == NDT - 1),
                    )
                nc.vector.tensor_scalar_add(QT[:, et, :], pq, bq_sb[:, et:et + 1])

            def mm2_chain(ET, QT, acc, kt):
                ps = psAccp.tile([P, QCH], F32, tag="acc")
                for et in range(NET):
                    nc.tensor.matmul(
                        ps,
                        keysT[:, et, kt * P:(kt + 1) * P],
                        QT[:, et, :],
                        start=(et == 0),
                        stop=(et == NET - 1),
                    )
                nc.scalar.activation(
                    ET[:, kt, :], ps, AFT.Exp,
                    bias=mask_sb[:, kt:kt + 1], scale=SCALE,
                )
                # denominator accumulation rides along on DVE
                if kt == 0:
                    nc.vector.tensor_copy(acc, ET[:, 0, :])
                else:
                    nc.vector.tensor_add(acc, acc, ET[:, kt, :])

            def sums_finish(acc):
                # ones^T @ acc -> [1, QCH]; scatter to [128, NQS] via DRAM
                # (an SBUF->SBUF partition-scatter AP silently corrupts on HW)
                pd = psDp.tile([1, QCH], F32, tag="psd")
                nc.tensor.matmul(pd, ones_f32, acc, start=True, stop=True)
                sums_sb = sumsp.tile([1, QCH], F32, tag="sums")
                nc.vector.tensor_copy(sums_sb, pd)
                scr = dramp.tile([1, QCH], F32, tag="scr")
                nc.sync.dma_start(scr[:, :], sums_sb)
                sums_t = sumsp.tile([P, NQS], F32, tag="sumst")
                nc.sync.dma_start(
                    sums_t, scr[:, :].rearrange("o (a p) -> p (o a)", p=P)
                )
                rc = sumsp.tile([P, NQS], F32, tag="rc")
                nc.vector.reciprocal(rc, sums_t)
                return rc

            def mm3_matmuls(ET, qs, dv):
                po = psAccp.tile([P, QCH], F32, tag="acc")
                for kt in range(NKT):
                    nc.tensor.matmul(
                        po,
                        ET[:, kt, qs * P:(qs + 1) * P],
                        Vsb[:, kt, dv * QCH:(dv + 1) * QCH],
                        start=(kt == 0),
                        stop=(kt == NKT - 1),
                    )
                return po

            def mm3_drain(qc, po, rc, qs, dv):
                osb = osbp.tile([P, QCH], F32, tag="osb")
                if (qs * 2 + dv) % 2 == 0:
                    nc.vector.tensor_scalar_mul(osb, po, rc[:, qs:qs + 1])
                else:
                    nc.scalar.activation(
                        osb, po, AFT.Copy, bias=0.0, scale=rc[:, qs:qs + 1],
                    )
                nc.sync.dma_start(
                    out_d[qc * QCH + qs * P: qc * QCH + (qs + 1) * P,
                          dv * QCH:(dv + 1) * QCH],
                    osb,
                )

            def v_stage_cvt(kt):
                st = stagep.tile([P, D], F32, tag="stage", name=f"vst{kt}")
                nc.sync.dma_start(st, values_d[kt * P:(kt + 1) * P, :])
                cvt = nc.vector.tensor_copy if kt % 2 == 0 else nc.scalar.copy
                cvt(Vsb[:, kt, :], st)

            # ---- emission ----
            # DMA priority: x chunk 0, consts, Wq, x chunk 1, keys; values are
            # emitted inside the MM2(0) loop (still ahead of x2/out stores).
            cv_x0 = [stage_rows(x_d[qs * P:(qs + 1) * P, :], qs)
                     for qs in range(NQS)]
            nc.sync.dma_start(bq_sb, bq_d[:].rearrange("(t p) -> p t", p=P))
            nc.sync.dma_start(
                mask_sb, mask_d[:].rearrange("(t p) o -> p (t o)", p=P)
            )
            cv_wq = [stage_rows(wq_d[et * P:(et + 1) * P, :], et)
                     for et in range(NDT)]
            cv_x1 = [stage_rows(x_d[QCH + qs * P:QCH + (qs + 1) * P, :], qs)
                     for qs in range(NQS)]
            cv_k = [stage_rows(keys_d[kt * P:(kt + 1) * P, :], kt)
                    for kt in range(NKT)]

            # PE: x0 transposes, then Wq transposes interleaved with MM1(0)
            xT0 = xTp.tile([P, NDT, QCH], BF16, tag="xT")
            for qs in range(NQS):
                transpose_block(xT0, qs * P, cv_x0[qs])
            QT0 = QTp.tile([P, NET, QCH], BF16, tag="QT")
            for et in range(NDT):
                transpose_block(WqT, et * P, cv_wq[et])
                mm1_chain(QT0, xT0, et)

            # x1 transposes, then keys transposes interleaved with MM1(1)
            xT1 = xTp.tile([P, NDT, QCH], BF16, tag="xT")
            for qs in range(NQS):
                transpose_block(xT1, qs * P, cv_x1[qs])
            QT1 = QTp.tile([P, NET, QCH], BF16, tag="QT")
            for kt in range(NKT):
                transpose_block(keysT, kt * P, cv_k[kt])
                if kt % 2 == 1:
                    mm1_chain(QT1, xT1, kt // 2)

            QTs = [QT0, QT1, None, None]
            xTs = [xT0, xT1, None, None]

            # steady state per chunk c: MM2(c) [+values cvt on c==0], MM3(c)
            # with the denominator finishing hidden behind the first MM3
            # chain, x(c+2) staging, MM1(c+2)
            for c in range(NQC):
                ET = ETp.tile([P, NKT, QCH], BF16, tag="ET", name=f"ET{c}")
                acc = accp.tile([P, QCH], F32, tag="accd", name=f"accd{c}")
                for kt in range(NKT):
                    mm2_chain(ET, QTs[c], acc, kt)
                    if c == 0:
                        v_stage_cvt(kt)
                cv_xn = None
                if c + 2 < NQC:
                    cv_xn = [stage_rows(
                        x_d[(c + 2) * QCH + qs * P:
                            (c + 2) * QCH + (qs + 1) * P, :], qs)
                        for qs in range(NQS)]
                    xTs[c + 2] = xTp.tile([P, NDT, QCH], BF16, tag="xT",
                                          name=f"xTn{c}")
                rc = None
                for qs in range(NQS):
                    if cv_xn is not None:
                        transpose_block(xTs[c + 2], qs * P, cv_xn[qs])
                    for dv in range(2):
                        po = mm3_matmuls(ET, qs, dv)
                        if rc is None:
                            # emitted after the first MM3 chain: the PE ones-
                            # matmul never waits on the DVE accumulate, and
                            # the DVE copy/recip precede every normalize in
                            # the DVE queue
                            rc = sums_finish(acc)
                        mm3_drain(c, po, rc, qs, dv)
                if c + 2 < NQC:
                    QTs[c + 2] = QTp.tile([P, NET, QCH], BF16, tag="QT",
                                          name=f"QTn{c}")
                    for et in range(NDT):
                        mm1_chain(QTs[c + 2], xTs[c + 2], et)

    nc.finalize()
    return nc


def _get_nc():
    if "nc" not in _CACHE:
        _CACHE["nc"] = build_nc()
    return _CACHE["nc"]


def kernel(x, mem_padding_mask, keys, values, Wq, bq):
    nc = _get_nc()
    Wq_c = np.ascontiguousarray(Wq, dtype=np.float32)
    bq_c = np.ascontiguousarray(bq, dtype=np.float32)
    in_maps = [
        {
            "x": np.ascontiguousarray(x[b], dtype=np.float32),
            "keys": np.ascontiguousarray(keys[b], dtype=np.float32),
            "values": np.ascontiguousarray(values[b], dtype=np.float32),
            "mask": np.ascontiguousarray(mem_padding_mask[b], dtype=np.float32),
            "Wq": Wq_c,
            "bq": bq_c,
        }
        for b in range(B)
    ]
    res = run_bass_kernel_spmd(nc, in_maps, core_ids=list(range(B)))
    return np.stack(
        [np.asarray(res.results[i]["out"]) for i in range(B)], axis=0
    ).astype(np.float32)
